# revision 2
# baseline (speedup 1.0000x reference)
"""GNN message-passing block (edge MLP + scatter-mean + node update MLP
+ masked residual LayerNorm) on 8 Trainium2 NeuronCores.

Strategy:
  - Edges sorted by destination node; nodes split into 392 blocks of 128,
    49 blocks per core (dst-sharded => no cross-core reduction needed).
  - Per-core phase A: A = h @ W1a + (mb1 + C0), B = h @ W1b tables written
    to HBM scratch (C0 = emb[0] @ W1c folded into the bias; the edge-type
    term enters via a (C1-C0) row in the small feature matmul).
  - Edge phase, per 128-edge chunk: indirect-DMA row gathers A[src], B[dst];
    radial-basis features from host-precomputed distances; K=34 feature
    matmul; silu; mw2 matmul; silu; scatter-mean as a weighted one-hot
    matmul accumulating sums^T[h, d] in PSUM per block.
  - Node phase, per block: update MLP from sums^T, transpose, residual +
    LayerNorm + ligand mask, write output rows.

All 8 cores run an identical program (SPMD); per-block chunk counts are
padded to the max across cores at each block position.
"""

import sys

sys.path.insert(0, "/opt/trn_rl_repo")

import numpy as np
from concourse import bacc, bass, mybir
from concourse.tile import TileContext
from concourse.bass_utils import run_bass_kernel_spmd

F32 = mybir.dt.float32
I32 = mybir.dt.int32
AF = mybir.ActivationFunctionType
ALU = mybir.AluOpType

N = 50000
E = 800000
H = 128
R = 32
CUTOFF = 6.0
NCORE = 8
NB = 49                      # blocks per core
NBLK = NCORE * NB            # 392
NPAD = NBLK * 128            # 50176
GAMMA = 1.0 / max((CUTOFF / (R - 1)) ** 2, 1e-6)
LN_EPS = 1e-5

_cache = {}


def _build(kc):
    """Emit the SPMD Bacc program. kc: tuple of chunks per block position."""
    tot = sum(kc)
    nc = bacc.Bacc()

    hT = nc.declare_dram_parameter("hT", [128, NPAD], F32, isOutput=False)
    h_own = nc.declare_dram_parameter("h_own", [NB * 128, H], F32, isOutput=False)
    hTown = nc.declare_dram_parameter("hTown", [128, NB * 128], F32, isOutput=False)
    w1a = nc.declare_dram_parameter("w1a", [H, H], F32, isOutput=False)
    w1b = nc.declare_dram_parameter("w1b", [H, H], F32, isOutput=False)
    wfeat = nc.declare_dram_parameter("wfeat", [34, H], F32, isOutput=False)
    mw2 = nc.declare_dram_parameter("mw2", [H, H], F32, isOutput=False)
    utop = nc.declare_dram_parameter("utop", [H, H], F32, isOutput=False)
    ubot = nc.declare_dram_parameter("ubot", [H, H], F32, isOutput=False)
    uw2 = nc.declare_dram_parameter("uw2", [H, H], F32, isOutput=False)
    ub1 = nc.declare_dram_parameter("ub1", [H, 1], F32, isOutput=False)
    ub2 = nc.declare_dram_parameter("ub2", [H, 1], F32, isOutput=False)
    mb1c0 = nc.declare_dram_parameter("mb1c0", [128, H], F32, isOutput=False)
    mb2rep = nc.declare_dram_parameter("mb2rep", [128, H], F32, isOutput=False)
    lngrep = nc.declare_dram_parameter("lngrep", [128, H], F32, isOutput=False)
    lnbrep = nc.declare_dram_parameter("lnbrep", [128, H], F32, isOutput=False)
    centers = nc.declare_dram_parameter("centers", [128, R], F32, isOutput=False)
    iota = nc.declare_dram_parameter("iota", [128, 128], F32, isOutput=False)
    ident = nc.declare_dram_parameter("ident", [128, 128], F32, isOutput=False)
    maskf = nc.declare_dram_parameter("maskf", [128, NB], F32, isOutput=False)
    esrc = nc.declare_dram_parameter("esrc", [128, tot], I32, isOutput=False)
    edst = nc.declare_dram_parameter("edst", [128, tot], I32, isOutput=False)
    edata = nc.declare_dram_parameter("edata", [128, 4 * tot], F32, isOutput=False)
    out = nc.declare_dram_parameter("out", [NB * 128, H], F32, isOutput=True)

    A_hbm = nc.dram_tensor("A_scr", [NPAD, H], F32)
    B_hbm = nc.dram_tensor("B_scr", [NPAD, H], F32)

    with TileContext(nc) as tc:
        with (
            tc.tile_pool(name="pc", bufs=1) as pc,
            tc.tile_pool(name="pa", bufs=3) as pa,
            tc.tile_pool(name="pb", bufs=2) as pb,
            tc.tile_pool(name="pw", bufs=2) as pw,
            tc.tile_pool(name="pps", bufs=6, space="PSUM") as pps,
            tc.tile_pool(name="psums", bufs=2, space="PSUM") as psums,
        ):
            def cload(ap, shape, tag, dtype=F32):
                t = pc.tile(shape, dtype, tag=tag)
                nc.sync.dma_start(out=t[:], in_=ap[:])
                return t

            w1a_t = cload(w1a, [H, H], "w1a")
            w1b_t = cload(w1b, [H, H], "w1b")
            wfeat_t = cload(wfeat, [34, H], "wfeat")
            mw2_t = cload(mw2, [H, H], "mw2")
            utop_t = cload(utop, [H, H], "utop")
            ubot_t = cload(ubot, [H, H], "ubot")
            uw2_t = cload(uw2, [H, H], "uw2")
            ub1_t = cload(ub1, [H, 1], "ub1")
            ub2_t = cload(ub2, [H, 1], "ub2")
            mb1c0_t = cload(mb1c0, [128, H], "mb1c0")
            mb2_t = cload(mb2rep, [128, H], "mb2")
            lng_t = cload(lngrep, [128, H], "lng")
            lnb_t = cload(lnbrep, [128, H], "lnb")
            cen_t = cload(centers, [128, R], "cen")
            iota_t = cload(iota, [128, 128], "iota")
            id_t = cload(ident, [128, 128], "ident")
            mask_t = cload(maskf, [128, NB], "maskf")
            hTown_t = cload(hTown, [128, NB * 128], "hTown")

            # ---- phase A: A/B tables ----
            for cn in range(NBLK):
                sl = slice(cn * 128, (cn + 1) * 128)
                hTc = pa.tile([128, 128], F32, tag="hTc")
                nc.sync.dma_start(out=hTc[:], in_=hT[:, sl])
                pA = pps.tile([128, H], F32, tag="ps")
                nc.tensor.matmul(pA[:], hTc[:], w1a_t[:], start=True, stop=True)
                aS = pa.tile([128, H], F32, tag="aS")
                nc.vector.tensor_tensor(aS[:], pA[:], mb1c0_t[:], op=ALU.add)
                nc.sync.dma_start(out=A_hbm[sl, :], in_=aS[:])
                pB = pps.tile([128, H], F32, tag="ps")
                nc.tensor.matmul(pB[:], hTc[:], w1b_t[:], start=True, stop=True)
                bS = pa.tile([128, H], F32, tag="bS")
                nc.vector.tensor_copy(bS[:], pB[:])
                nc.sync.dma_start(out=B_hbm[sl, :], in_=bS[:])

            tc.strict_bb_all_engine_barrier()

            # ---- edge + node phases, per block ----
            q0 = 0
            for j in range(NB):
                kcj = kc[j]
                esb = pb.tile([128, kcj], I32, tag="esb")
                nc.sync.dma_start(out=esb[:], in_=esrc[:, q0 : q0 + kcj])
                edb = pb.tile([128, kcj], I32, tag="edb")
                nc.sync.dma_start(out=edb[:], in_=edst[:, q0 : q0 + kcj])
                eab = pb.tile([128, 4 * kcj], F32, tag="eab")
                nc.sync.dma_start(
                    out=eab[:], in_=edata[:, 4 * q0 : 4 * (q0 + kcj)]
                )
                sums = psums.tile([128, 128], F32, tag="sums")

                for k in range(kcj):
                    ga = pw.tile([128, H], F32, tag="ga")
                    nc.gpsimd.indirect_dma_start(
                        out=ga[:], out_offset=None, in_=A_hbm[:],
                        in_offset=bass.IndirectOffsetOnAxis(
                            ap=esb[:, k : k + 1], axis=0),
                    )
                    gb = pw.tile([128, H], F32, tag="gb")
                    nc.gpsimd.indirect_dma_start(
                        out=gb[:], out_offset=None, in_=B_hbm[:],
                        in_offset=bass.IndirectOffsetOnAxis(
                            ap=edb[:, k : k + 1], axis=0),
                    )
                    dist_ap = eab[:, 4 * k + 2 : 4 * k + 3]
                    sq = pw.tile([128, R], F32, tag="sq")
                    nc.scalar.activation(sq[:], cen_t[:], AF.Square,
                                         bias=dist_ap, scale=-1.0)
                    ft = pw.tile([128, 34], F32, tag="ft")
                    nc.scalar.activation(ft[:, 0:R], sq[:], AF.Exp, scale=-GAMMA)
                    nc.vector.tensor_copy(ft[:, R : R + 2],
                                          eab[:, 4 * k + 2 : 4 * k + 4])
                    fT = pps.tile([34, 128], F32, tag="ps")
                    nc.tensor.transpose(fT[:], ft[:], id_t[:])
                    fTs = pw.tile([34, 128], F32, tag="fTs")
                    nc.vector.tensor_copy(fTs[:], fT[:])
                    xps = pps.tile([128, H], F32, tag="ps")
                    nc.tensor.matmul(xps[:], fTs[:], wfeat_t[:],
                                     start=True, stop=True)
                    xs = pw.tile([128, H], F32, tag="xs")
                    nc.vector.tensor_tensor(xs[:], ga[:], gb[:], op=ALU.add)
                    nc.vector.tensor_tensor(xs[:], xs[:], xps[:], op=ALU.add)
                    xsl = pw.tile([128, H], F32, tag="xsl")
                    nc.scalar.activation(xsl[:], xs[:], AF.Silu)
                    xT = pps.tile([128, 128], F32, tag="ps")
                    nc.tensor.transpose(xT[:], xsl[:], id_t[:])
                    xTs = pw.tile([128, 128], F32, tag="xTs")
                    nc.vector.tensor_copy(xTs[:], xT[:])
                    yps = pps.tile([128, H], F32, tag="ps")
                    nc.tensor.matmul(yps[:], xTs[:], mw2_t[:],
                                     start=True, stop=True)
                    ms = pw.tile([128, H], F32, tag="ms")
                    nc.vector.tensor_tensor(ms[:], yps[:], mb2_t[:], op=ALU.add)
                    ms2 = pw.tile([128, H], F32, tag="ms2")
                    nc.scalar.activation(ms2[:], ms[:], AF.Silu)
                    ohw = pw.tile([128, 128], F32, tag="ohw")
                    nc.vector.tensor_scalar(
                        ohw[:], iota_t[:], eab[:, 4 * k : 4 * k + 1],
                        eab[:, 4 * k + 1 : 4 * k + 2],
                        ALU.is_equal, ALU.mult,
                    )
                    nc.tensor.matmul(sums[:], ms2[:], ohw[:],
                                     start=(k == 0), stop=(k == kcj - 1))

                # ---- node update for this block ----
                aggT = pw.tile([128, 128], F32, tag="aggT")
                nc.vector.tensor_copy(aggT[:], sums[:])
                ups = pps.tile([128, 128], F32, tag="ps")
                nc.tensor.matmul(ups[:], utop_t[:],
                                 hTown_t[:, j * 128 : (j + 1) * 128],
                                 start=True, stop=False)
                nc.tensor.matmul(ups[:], ubot_t[:], aggT[:],
                                 start=False, stop=True)
                us = pw.tile([128, 128], F32, tag="us")
                nc.scalar.activation(us[:], ups[:], AF.Silu, bias=ub1_t[:, 0:1])
                uds = pps.tile([128, 128], F32, tag="ps")
                nc.tensor.matmul(uds[:], uw2_t[:], us[:], start=True, stop=True)
                udb = pw.tile([128, 128], F32, tag="udb")
                nc.vector.tensor_scalar(udb[:], uds[:], ub2_t[:, 0:1], None,
                                        ALU.add)
                updp = pps.tile([128, 128], F32, tag="ps")
                nc.tensor.transpose(updp[:], udb[:], id_t[:])
                hb = pb.tile([128, 128], F32, tag="hb")
                nc.sync.dma_start(out=hb[:],
                                  in_=h_own[j * 128 : (j + 1) * 128, :])
                z = pw.tile([128, H], F32, tag="z")
                nc.vector.tensor_tensor(z[:], updp[:], hb[:], op=ALU.add)
                mu = pw.tile([128, 1], F32, tag="mu")
                nc.vector.tensor_reduce(mu[:], z[:], mybir.AxisListType.X,
                                        ALU.add)
                nc.vector.tensor_scalar(mu[:], mu[:], 1.0 / H, None, ALU.mult)
                zc = pw.tile([128, H], F32, tag="zc")
                nc.vector.tensor_scalar(zc[:], z[:], mu[:, 0:1], None,
                                        ALU.subtract)
                sqd = pw.tile([128, H], F32, tag="sqd")
                ss = pw.tile([128, 1], F32, tag="ss")
                nc.scalar.activation(sqd[:], zc[:], AF.Square, accum_out=ss[:])
                ra = pw.tile([128, 1], F32, tag="ra")
                nc.vector.tensor_scalar(ra[:], ss[:], 1.0 / H, LN_EPS,
                                        ALU.mult, ALU.add)
                sd = pw.tile([128, 1], F32, tag="sd")
                nc.scalar.activation(sd[:], ra[:], AF.Sqrt)
                rs = pw.tile([128, 1], F32, tag="rs")
                nc.vector.reciprocal(rs[:], sd[:])
                nm = pw.tile([128, H], F32, tag="nm")
                nc.vector.tensor_scalar(nm[:], zc[:], rs[:, 0:1], None, ALU.mult)
                nc.vector.tensor_tensor(nm[:], nm[:], lng_t[:], op=ALU.mult)
                nc.vector.tensor_tensor(nm[:], nm[:], lnb_t[:], op=ALU.add)
                d1 = pw.tile([128, H], F32, tag="d1")
                nc.vector.tensor_tensor(d1[:], nm[:], hb[:], op=ALU.subtract)
                nc.vector.tensor_scalar(d1[:], d1[:], mask_t[:, j : j + 1],
                                        None, ALU.mult)
                nc.vector.tensor_tensor(d1[:], d1[:], hb[:], op=ALU.add)
                nc.sync.dma_start(out=out[j * 128 : (j + 1) * 128, :], in_=d1[:])
                q0 += kcj

    nc.compile()
    return nc


def _prep(h, pos, edge_index, edge_type, node_type,
          emb, mw1, mb1, mw2, mb2, uw1, ub1, uw2, ub2, ln_g, ln_b):
    h = np.asarray(h, np.float32)
    pos = np.asarray(pos, np.float32)
    src = np.asarray(edge_index[0], np.int64)
    dst = np.asarray(edge_index[1], np.int64)
    et = np.asarray(edge_type, np.int64)
    ntype = np.asarray(node_type)
    mw1 = np.asarray(mw1, np.float32)
    emb = np.asarray(emb, np.float32)

    blk = dst >> 7
    order = np.lexsort((src, blk))
    src_s = src[order]
    dst_s = dst[order]
    blk_s = blk[order]
    cnt = np.bincount(dst, minlength=N).astype(np.float32)
    w_s = (1.0 / np.maximum(cnt, 1.0))[dst_s].astype(np.float32)
    rel = pos[src_s] - pos[dst_s]
    dist_s = np.sqrt((rel * rel).sum(axis=1)).astype(np.float32)
    dl_s = (dst_s & 127).astype(np.float32)
    et_s = et[order].astype(np.float32)

    bc = np.bincount(blk_s, minlength=NBLK)
    bstart = np.zeros(NBLK + 1, np.int64)
    np.cumsum(bc, out=bstart[1:])
    cnts = bc.reshape(NCORE, NB)
    kc = np.maximum(1, (cnts + 127) // 128).max(axis=0)
    tot = int(kc.sum())

    per_core = []
    for c in range(NCORE):
        fsrc = np.zeros(tot * 128, np.int32)
        fdst = np.zeros(tot * 128, np.int32)
        fdat = np.zeros((tot * 128, 4), np.float32)
        fdat[:, 2] = 1.0  # dummy dist, benign
        base = 0
        for j in range(NB):
            g = c * NB + j
            s0, s1 = bstart[g], bstart[g + 1]
            n = s1 - s0
            fsrc[base : base + n] = src_s[s0:s1]
            fdst[base : base + n] = dst_s[s0:s1]
            fdat[base : base + n, 0] = dl_s[s0:s1]
            fdat[base : base + n, 1] = w_s[s0:s1]
            fdat[base : base + n, 2] = dist_s[s0:s1]
            fdat[base : base + n, 3] = et_s[s0:s1]
            base += int(kc[j]) * 128
        esrc2 = fsrc.reshape(tot, 128).T.copy()
        edst2 = fdst.reshape(tot, 128).T.copy()
        edata2 = np.ascontiguousarray(
            fdat.reshape(tot, 128, 4).transpose(1, 0, 2).reshape(128, 4 * tot))
        per_core.append((esrc2, edst2, edata2))

    hT = np.zeros((128, NPAD), np.float32)
    hT[:, :N] = h.T
    h_pad = np.zeros((NPAD, H), np.float32)
    h_pad[:N] = h
    maskp = np.zeros(NPAD, np.float32)
    maskp[:N] = (np.asarray(ntype) == 0).astype(np.float32)

    W1a = np.ascontiguousarray(mw1[0:128])
    W1b = np.ascontiguousarray(mw1[128:256])
    W1c = mw1[256:384]
    W1d = mw1[384:416]
    w1e = mw1[416:417]
    C = emb @ W1c  # [2, H]
    wfeat = np.ascontiguousarray(
        np.vstack([W1d, w1e, (C[1] - C[0])[None, :]]).astype(np.float32))
    mb1c0 = np.tile((np.asarray(mb1, np.float32) + C[0])[None, :], (128, 1))
    mb2rep = np.tile(np.asarray(mb2, np.float32)[None, :], (128, 1))
    lngrep = np.tile(np.asarray(ln_g, np.float32)[None, :], (128, 1))
    lnbrep = np.tile(np.asarray(ln_b, np.float32)[None, :], (128, 1))
    cen = np.tile(np.linspace(0.0, CUTOFF, R, dtype=np.float32)[None, :],
                  (128, 1))
    iota = np.tile(np.arange(128, dtype=np.float32)[None, :], (128, 1))
    ident = np.eye(128, dtype=np.float32)
    uw1 = np.asarray(uw1, np.float32)

    shared = {
        "hT": hT,
        "w1a": W1a, "w1b": W1b, "wfeat": wfeat,
        "mw2": np.asarray(mw2, np.float32),
        "utop": np.ascontiguousarray(uw1[0:128]),
        "ubot": np.ascontiguousarray(uw1[128:256]),
        "uw2": np.asarray(uw2, np.float32),
        "ub1": np.asarray(ub1, np.float32).reshape(H, 1),
        "ub2": np.asarray(ub2, np.float32).reshape(H, 1),
        "mb1c0": np.ascontiguousarray(mb1c0),
        "mb2rep": np.ascontiguousarray(mb2rep),
        "lngrep": np.ascontiguousarray(lngrep),
        "lnbrep": np.ascontiguousarray(lnbrep),
        "centers": np.ascontiguousarray(cen),
        "iota": np.ascontiguousarray(iota),
        "ident": ident,
    }
    in_maps = []
    for c in range(NCORE):
        esrc2, edst2, edata2 = per_core[c]
        rows = slice(c * NB * 128, (c + 1) * NB * 128)
        m = dict(shared)
        m["h_own"] = np.ascontiguousarray(h_pad[rows])
        m["hTown"] = np.ascontiguousarray(hT[:, rows])
        m["maskf"] = np.ascontiguousarray(
            maskp[rows].reshape(NB, 128).T)
        m["esrc"] = esrc2
        m["edst"] = edst2
        m["edata"] = edata2
        in_maps.append(m)
    return tuple(int(x) for x in kc), in_maps


def kernel(**inputs):
    kc, in_maps = _prep(**inputs)
    if kc not in _cache:
        _cache[kc] = _build(kc)
    nc = _cache[kc]
    res = run_bass_kernel_spmd(nc, in_maps, list(range(NCORE)))
    outs = [res.results[c]["out"] for c in range(NCORE)]
    full = np.concatenate(outs, axis=0)[:N]
    return np.ascontiguousarray(full.astype(np.float32))


# revision 3
# speedup vs baseline: 1310.5452x; 1310.5452x over previous
"""GNN message-passing block (edge MLP + scatter-mean + node update MLP
+ masked residual LayerNorm) on 8 Trainium2 NeuronCores.

Strategy:
  - Edges sorted by destination node; nodes split into 392 blocks of 128,
    49 blocks per core (dst-sharded => no cross-core reduction needed).
  - Per-core phase A: A = h @ W1a + (mb1 + C0), B = h @ W1b tables written
    to HBM scratch (C0 = emb[0] @ W1c folded into the bias; the edge-type
    term enters via a (C1-C0) row in the small feature matmul).
  - Edge phase, per 128-edge chunk: indirect-DMA row gathers A[src], B[dst];
    radial-basis features from host-precomputed distances; K=34 feature
    matmul; silu; mw2 matmul; silu; scatter-mean as a weighted one-hot
    matmul accumulating sums^T[h, d] in PSUM per block.
  - Node phase, per block: update MLP from sums^T, transpose, residual +
    LayerNorm + ligand mask, write output rows.

All 8 cores run an identical program (SPMD); per-block chunk counts are
padded to the max across cores at each block position.
"""

import sys

sys.path.insert(0, "/opt/trn_rl_repo")

import numpy as np
from concourse import bacc, bass, mybir
from concourse.tile import TileContext
from concourse.bass_utils import run_bass_kernel_spmd

F32 = mybir.dt.float32
I32 = mybir.dt.int32
AF = mybir.ActivationFunctionType
ALU = mybir.AluOpType

N = 50000
E = 800000
H = 128
R = 32
CUTOFF = 6.0
NCORE = 8
NB = 49                      # blocks per core
NBLK = NCORE * NB            # 392
NPAD = NBLK * 128            # 50176
GAMMA = 1.0 / max((CUTOFF / (R - 1)) ** 2, 1e-6)
LN_EPS = 1e-5

_cache = {}


def _build(kc):
    """Emit the SPMD Bacc program. kc: tuple of chunks per block position."""
    tot = sum(kc)
    nc = bacc.Bacc()

    hT = nc.declare_dram_parameter("hT", [128, NPAD], F32, isOutput=False)
    h_own = nc.declare_dram_parameter("h_own", [NB * 128, H], F32, isOutput=False)
    hTown = nc.declare_dram_parameter("hTown", [128, NB * 128], F32, isOutput=False)
    w1a = nc.declare_dram_parameter("w1a", [H, H], F32, isOutput=False)
    w1b = nc.declare_dram_parameter("w1b", [H, H], F32, isOutput=False)
    wfeat = nc.declare_dram_parameter("wfeat", [34, H], F32, isOutput=False)
    mw2 = nc.declare_dram_parameter("mw2", [H, H], F32, isOutput=False)
    utop = nc.declare_dram_parameter("utop", [H, H], F32, isOutput=False)
    ubot = nc.declare_dram_parameter("ubot", [H, H], F32, isOutput=False)
    uw2 = nc.declare_dram_parameter("uw2", [H, H], F32, isOutput=False)
    ub1 = nc.declare_dram_parameter("ub1", [H, 1], F32, isOutput=False)
    ub2 = nc.declare_dram_parameter("ub2", [H, 1], F32, isOutput=False)
    mb1c0 = nc.declare_dram_parameter("mb1c0", [128, H], F32, isOutput=False)
    mb2rep = nc.declare_dram_parameter("mb2rep", [128, H], F32, isOutput=False)
    lngrep = nc.declare_dram_parameter("lngrep", [128, H], F32, isOutput=False)
    lnbrep = nc.declare_dram_parameter("lnbrep", [128, H], F32, isOutput=False)
    centers = nc.declare_dram_parameter("centers", [128, R], F32, isOutput=False)
    iota = nc.declare_dram_parameter("iota", [128, 128], F32, isOutput=False)
    ident = nc.declare_dram_parameter("ident", [128, 128], F32, isOutput=False)
    maskf = nc.declare_dram_parameter("maskf", [128, NB], F32, isOutput=False)
    esrc = nc.declare_dram_parameter("esrc", [128, tot], I32, isOutput=False)
    edst = nc.declare_dram_parameter("edst", [128, tot], I32, isOutput=False)
    edata = nc.declare_dram_parameter("edata", [128, 4 * tot], F32, isOutput=False)
    out = nc.declare_dram_parameter("out", [NB * 128, H], F32, isOutput=True)

    A_hbm = nc.dram_tensor("A_scr", [NPAD, H], F32)
    B_hbm = nc.dram_tensor("B_scr", [NPAD, H], F32)

    with TileContext(nc) as tc:
        with (
            tc.tile_pool(name="pc", bufs=1) as pc,
            tc.tile_pool(name="pa", bufs=3) as pa,
            tc.tile_pool(name="pb", bufs=2) as pb,
            tc.tile_pool(name="pw", bufs=2) as pw,
            tc.tile_pool(name="pps", bufs=6, space="PSUM") as pps,
            tc.tile_pool(name="psums", bufs=2, space="PSUM") as psums,
        ):
            def cload(ap, shape, tag, dtype=F32):
                t = pc.tile(shape, dtype, tag=tag)
                nc.sync.dma_start(out=t[:], in_=ap[:])
                return t

            w1a_t = cload(w1a, [H, H], "w1a")
            w1b_t = cload(w1b, [H, H], "w1b")
            wfeat_t = cload(wfeat, [34, H], "wfeat")
            mw2_t = cload(mw2, [H, H], "mw2")
            utop_t = cload(utop, [H, H], "utop")
            ubot_t = cload(ubot, [H, H], "ubot")
            uw2_t = cload(uw2, [H, H], "uw2")
            ub1_t = cload(ub1, [H, 1], "ub1")
            ub2_t = cload(ub2, [H, 1], "ub2")
            mb1c0_t = cload(mb1c0, [128, H], "mb1c0")
            mb2_t = cload(mb2rep, [128, H], "mb2")
            lng_t = cload(lngrep, [128, H], "lng")
            lnb_t = cload(lnbrep, [128, H], "lnb")
            cen_t = cload(centers, [128, R], "cen")
            iota_t = cload(iota, [128, 128], "iota")
            id_t = cload(ident, [128, 128], "ident")
            mask_t = cload(maskf, [128, NB], "maskf")
            hTown_t = cload(hTown, [128, NB * 128], "hTown")

            # ---- phase A: A/B tables ----
            for cn in range(NBLK):
                sl = slice(cn * 128, (cn + 1) * 128)
                hTc = pa.tile([128, 128], F32, tag="hTc")
                nc.sync.dma_start(out=hTc[:], in_=hT[:, sl])
                pA = pps.tile([128, H], F32, tag="ps")
                nc.tensor.matmul(pA[:], hTc[:], w1a_t[:], start=True, stop=True)
                aS = pa.tile([128, H], F32, tag="aS")
                nc.vector.tensor_tensor(aS[:], pA[:], mb1c0_t[:], op=ALU.add)
                nc.sync.dma_start(out=A_hbm[sl, :], in_=aS[:])
                pB = pps.tile([128, H], F32, tag="ps")
                nc.tensor.matmul(pB[:], hTc[:], w1b_t[:], start=True, stop=True)
                bS = pa.tile([128, H], F32, tag="bS")
                nc.vector.tensor_copy(bS[:], pB[:])
                nc.sync.dma_start(out=B_hbm[sl, :], in_=bS[:])

            tc.strict_bb_all_engine_barrier()

            # ---- edge + node phases, per block ----
            q0 = 0
            for j in range(NB):
                kcj = kc[j]
                esb = pb.tile([128, kcj], I32, tag="esb")
                nc.sync.dma_start(out=esb[:], in_=esrc[:, q0 : q0 + kcj])
                edb = pb.tile([128, kcj], I32, tag="edb")
                nc.sync.dma_start(out=edb[:], in_=edst[:, q0 : q0 + kcj])
                eab = pb.tile([128, 4 * kcj], F32, tag="eab")
                nc.sync.dma_start(
                    out=eab[:], in_=edata[:, 4 * q0 : 4 * (q0 + kcj)]
                )
                sums = psums.tile([128, 128], F32, tag="sums")

                for k in range(kcj):
                    ga = pw.tile([128, H], F32, tag="ga")
                    nc.gpsimd.indirect_dma_start(
                        out=ga[:], out_offset=None, in_=A_hbm[:],
                        in_offset=bass.IndirectOffsetOnAxis(
                            ap=esb[:, k : k + 1], axis=0),
                    )
                    gb = pw.tile([128, H], F32, tag="gb")
                    nc.gpsimd.indirect_dma_start(
                        out=gb[:], out_offset=None, in_=B_hbm[:],
                        in_offset=bass.IndirectOffsetOnAxis(
                            ap=edb[:, k : k + 1], axis=0),
                    )
                    dist_ap = eab[:, 4 * k + 2 : 4 * k + 3]
                    sq = pw.tile([128, R], F32, tag="sq")
                    nc.scalar.activation(sq[:], cen_t[:], AF.Square,
                                         bias=dist_ap, scale=-1.0)
                    ft = pw.tile([128, 34], F32, tag="ft")
                    nc.scalar.activation(ft[:, 0:R], sq[:], AF.Exp, scale=-GAMMA)
                    nc.vector.tensor_copy(ft[:, R : R + 2],
                                          eab[:, 4 * k + 2 : 4 * k + 4])
                    fT = pps.tile([34, 128], F32, tag="ps")
                    nc.tensor.transpose(fT[:], ft[:], id_t[:])
                    fTs = pw.tile([34, 128], F32, tag="fTs")
                    nc.vector.tensor_copy(fTs[:], fT[:])
                    xps = pps.tile([128, H], F32, tag="ps")
                    nc.tensor.matmul(xps[:], fTs[:], wfeat_t[:],
                                     start=True, stop=True)
                    xs = pw.tile([128, H], F32, tag="xs")
                    nc.vector.tensor_tensor(xs[:], ga[:], gb[:], op=ALU.add)
                    nc.vector.tensor_tensor(xs[:], xs[:], xps[:], op=ALU.add)
                    xsl = pw.tile([128, H], F32, tag="xsl")
                    nc.scalar.activation(xsl[:], xs[:], AF.Silu)
                    xT = pps.tile([128, 128], F32, tag="ps")
                    nc.tensor.transpose(xT[:], xsl[:], id_t[:])
                    xTs = pw.tile([128, 128], F32, tag="xTs")
                    nc.vector.tensor_copy(xTs[:], xT[:])
                    yps = pps.tile([128, H], F32, tag="ps")
                    nc.tensor.matmul(yps[:], xTs[:], mw2_t[:],
                                     start=True, stop=True)
                    ms = pw.tile([128, H], F32, tag="ms")
                    nc.vector.tensor_tensor(ms[:], yps[:], mb2_t[:], op=ALU.add)
                    ms2 = pw.tile([128, H], F32, tag="ms2")
                    nc.scalar.activation(ms2[:], ms[:], AF.Silu)
                    ohw = pw.tile([128, 128], F32, tag="ohw")
                    nc.vector.tensor_scalar(
                        ohw[:], iota_t[:], eab[:, 4 * k : 4 * k + 1],
                        eab[:, 4 * k + 1 : 4 * k + 2],
                        ALU.is_equal, ALU.mult,
                    )
                    nc.tensor.matmul(sums[:], ms2[:], ohw[:],
                                     start=(k == 0), stop=(k == kcj - 1))

                # ---- node update for this block ----
                aggT = pw.tile([128, 128], F32, tag="aggT")
                nc.vector.tensor_copy(aggT[:], sums[:])
                ups = pps.tile([128, 128], F32, tag="ps")
                nc.tensor.matmul(ups[:], utop_t[:],
                                 hTown_t[:, j * 128 : (j + 1) * 128],
                                 start=True, stop=False)
                nc.tensor.matmul(ups[:], ubot_t[:], aggT[:],
                                 start=False, stop=True)
                us = pw.tile([128, 128], F32, tag="us")
                nc.scalar.activation(us[:], ups[:], AF.Silu, bias=ub1_t[:, 0:1])
                uds = pps.tile([128, 128], F32, tag="ps")
                nc.tensor.matmul(uds[:], uw2_t[:], us[:], start=True, stop=True)
                udb = pw.tile([128, 128], F32, tag="udb")
                nc.vector.tensor_scalar(udb[:], uds[:], ub2_t[:, 0:1], None,
                                        ALU.add)
                updp = pps.tile([128, 128], F32, tag="ps")
                nc.tensor.transpose(updp[:], udb[:], id_t[:])
                hb = pb.tile([128, 128], F32, tag="hb")
                nc.sync.dma_start(out=hb[:],
                                  in_=h_own[j * 128 : (j + 1) * 128, :])
                z = pw.tile([128, H], F32, tag="z")
                nc.vector.tensor_tensor(z[:], updp[:], hb[:], op=ALU.add)
                mu = pw.tile([128, 1], F32, tag="mu")
                nc.vector.tensor_reduce(mu[:], z[:], mybir.AxisListType.X,
                                        ALU.add)
                nc.vector.tensor_scalar(mu[:], mu[:], 1.0 / H, None, ALU.mult)
                zc = pw.tile([128, H], F32, tag="zc")
                nc.vector.tensor_scalar(zc[:], z[:], mu[:, 0:1], None,
                                        ALU.subtract)
                sqd = pw.tile([128, H], F32, tag="sqd")
                ss = pw.tile([128, 1], F32, tag="ss")
                nc.scalar.activation(sqd[:], zc[:], AF.Square, accum_out=ss[:])
                ra = pw.tile([128, 1], F32, tag="ra")
                nc.vector.tensor_scalar(ra[:], ss[:], 1.0 / H, LN_EPS,
                                        ALU.mult, ALU.add)
                sd = pw.tile([128, 1], F32, tag="sd")
                nc.scalar.activation(sd[:], ra[:], AF.Sqrt)
                rs = pw.tile([128, 1], F32, tag="rs")
                nc.vector.reciprocal(rs[:], sd[:])
                nm = pw.tile([128, H], F32, tag="nm")
                nc.vector.tensor_scalar(nm[:], zc[:], rs[:, 0:1], None, ALU.mult)
                nc.vector.tensor_tensor(nm[:], nm[:], lng_t[:], op=ALU.mult)
                nc.vector.tensor_tensor(nm[:], nm[:], lnb_t[:], op=ALU.add)
                d1 = pw.tile([128, H], F32, tag="d1")
                nc.vector.tensor_tensor(d1[:], nm[:], hb[:], op=ALU.subtract)
                nc.vector.tensor_scalar(d1[:], d1[:], mask_t[:, j : j + 1],
                                        None, ALU.mult)
                nc.vector.tensor_tensor(d1[:], d1[:], hb[:], op=ALU.add)
                nc.sync.dma_start(out=out[j * 128 : (j + 1) * 128, :], in_=d1[:])
                q0 += kcj

    nc.compile()
    return nc


def _prep(h, pos, edge_index, edge_type, node_type,
          emb, mw1, mb1, mw2, mb2, uw1, ub1, uw2, ub2, ln_g, ln_b):
    h = np.asarray(h, np.float32)
    pos = np.asarray(pos, np.float32)
    src = np.asarray(edge_index[0], np.int64)
    dst = np.asarray(edge_index[1], np.int64)
    et = np.asarray(edge_type, np.int64)
    ntype = np.asarray(node_type)
    mw1 = np.asarray(mw1, np.float32)
    emb = np.asarray(emb, np.float32)

    blk = dst >> 7
    order = np.lexsort((src, blk))
    src_s = src[order]
    dst_s = dst[order]
    blk_s = blk[order]
    cnt = np.bincount(dst, minlength=N).astype(np.float32)
    w_s = (1.0 / np.maximum(cnt, 1.0))[dst_s].astype(np.float32)
    rel = pos[src_s] - pos[dst_s]
    dist_s = np.sqrt((rel * rel).sum(axis=1)).astype(np.float32)
    dl_s = (dst_s & 127).astype(np.float32)
    et_s = et[order].astype(np.float32)

    bc = np.bincount(blk_s, minlength=NBLK)
    bstart = np.zeros(NBLK + 1, np.int64)
    np.cumsum(bc, out=bstart[1:])
    cnts = bc.reshape(NCORE, NB)
    kc = np.maximum(1, (cnts + 127) // 128).max(axis=0)
    tot = int(kc.sum())

    per_core = []
    for c in range(NCORE):
        fsrc = np.zeros(tot * 128, np.int32)
        fdst = np.zeros(tot * 128, np.int32)
        fdat = np.zeros((tot * 128, 4), np.float32)
        fdat[:, 2] = 1.0  # dummy dist, benign
        base = 0
        for j in range(NB):
            g = c * NB + j
            s0, s1 = bstart[g], bstart[g + 1]
            n = s1 - s0
            fsrc[base : base + n] = src_s[s0:s1]
            fdst[base : base + n] = dst_s[s0:s1]
            fdat[base : base + n, 0] = dl_s[s0:s1]
            fdat[base : base + n, 1] = w_s[s0:s1]
            fdat[base : base + n, 2] = dist_s[s0:s1]
            fdat[base : base + n, 3] = et_s[s0:s1]
            base += int(kc[j]) * 128
        esrc2 = fsrc.reshape(tot, 128).T.copy()
        edst2 = fdst.reshape(tot, 128).T.copy()
        edata2 = np.ascontiguousarray(
            fdat.reshape(tot, 128, 4).transpose(1, 0, 2).reshape(128, 4 * tot))
        per_core.append((esrc2, edst2, edata2))

    hT = np.zeros((128, NPAD), np.float32)
    hT[:, :N] = h.T
    h_pad = np.zeros((NPAD, H), np.float32)
    h_pad[:N] = h
    maskp = np.zeros(NPAD, np.float32)
    maskp[:N] = (np.asarray(ntype) == 0).astype(np.float32)

    W1a = np.ascontiguousarray(mw1[0:128])
    W1b = np.ascontiguousarray(mw1[128:256])
    W1c = mw1[256:384]
    W1d = mw1[384:416]
    w1e = mw1[416:417]
    C = emb @ W1c  # [2, H]
    wfeat = np.ascontiguousarray(
        np.vstack([W1d, w1e, (C[1] - C[0])[None, :]]).astype(np.float32))
    mb1c0 = np.tile((np.asarray(mb1, np.float32) + C[0])[None, :], (128, 1))
    mb2rep = np.tile(np.asarray(mb2, np.float32)[None, :], (128, 1))
    lngrep = np.tile(np.asarray(ln_g, np.float32)[None, :], (128, 1))
    lnbrep = np.tile(np.asarray(ln_b, np.float32)[None, :], (128, 1))
    cen = np.tile(np.linspace(0.0, CUTOFF, R, dtype=np.float32)[None, :],
                  (128, 1))
    iota = np.tile(np.arange(128, dtype=np.float32)[None, :], (128, 1))
    ident = np.eye(128, dtype=np.float32)
    uw1 = np.asarray(uw1, np.float32)

    shared = {
        "hT": hT,
        "w1a": W1a, "w1b": W1b, "wfeat": wfeat,
        "mw2": np.asarray(mw2, np.float32),
        "utop": np.ascontiguousarray(uw1[0:128]),
        "ubot": np.ascontiguousarray(uw1[128:256]),
        "uw2": np.asarray(uw2, np.float32),
        "ub1": np.asarray(ub1, np.float32).reshape(H, 1),
        "ub2": np.asarray(ub2, np.float32).reshape(H, 1),
        "mb1c0": np.ascontiguousarray(mb1c0),
        "mb2rep": np.ascontiguousarray(mb2rep),
        "lngrep": np.ascontiguousarray(lngrep),
        "lnbrep": np.ascontiguousarray(lnbrep),
        "centers": np.ascontiguousarray(cen),
        "iota": np.ascontiguousarray(iota),
        "ident": ident,
    }
    in_maps = []
    for c in range(NCORE):
        esrc2, edst2, edata2 = per_core[c]
        rows = slice(c * NB * 128, (c + 1) * NB * 128)
        m = dict(shared)
        m["h_own"] = np.ascontiguousarray(h_pad[rows])
        m["hTown"] = np.ascontiguousarray(hT[:, rows])
        m["maskf"] = np.ascontiguousarray(
            maskp[rows].reshape(NB, 128).T)
        m["esrc"] = esrc2
        m["edst"] = edst2
        m["edata"] = edata2
        in_maps.append(m)
    return tuple(int(x) for x in kc), in_maps


def kernel(**inputs):
    res = kernel_raw(**inputs)
    outs = [res.results[c]["out"] for c in range(NCORE)]
    full = np.concatenate(outs, axis=0)[:N]
    return np.ascontiguousarray(full.astype(np.float32))


def kernel_raw(_trace=False, **inputs):
    kc, in_maps = _prep(**inputs)
    if kc not in _cache:
        _cache[kc] = _build(kc)
    nc = _cache[kc]
    return run_bass_kernel_spmd(nc, in_maps, list(range(NCORE)), trace=_trace)
